# revision 1
# baseline (speedup 1.0000x reference)
"""Trainium2 Bass kernel for nn_BezierGlyph (retrieval_knn).

Math (matching the jax reference):
  pts  = cubic-bezier samples of clip(control_points, 0, 1)   # [512, 2]
  d_ij = |pixel_i - pts_j|
  m_i  = -logsumexp(-256 * d_i:) / 256                        # softmin
  out  = 1 - sigmoid((0.04 - m) * 200)                        # (1, 512, 512)

Strategy (sharding_hint: shard pixels, replicate points):
  * 512x512 pixels split into 256 blocks of 32x32; each block only needs
    sample points within 0.2 of its bbox (farther points contribute less
    than 1e-5 relative to the softmin sum wherever the output is not
    exactly 1.0f; dropping them only biases far-pixel sums DOWN, which
    keeps those outputs at exactly 1.0f).
  * Blocks are LPT-balanced across the 8 cores (32 blocks each). The SPMD
    program is shared, so per-slot candidate capacity K_sched[i] is the
    max across cores of each core's i-th largest padded candidate count.
  * dist^2 = |p|^2 - 2 p.q + |q|^2 via one PE matmul with an 18-row bf16
    contraction: each fp32 factor is split into 3 bf16 limbs (exact),
    bf16xbf16 products are exact in the fp32 PSUM accumulator, and limb
    products below 2^-24 are dropped. 4x faster than fp32 matmul at the
    same effective precision.
  * Scalar engine uses a single activation-table set
    (natural_log_exp_and_others; a post-compile pass dedups the
    per-function table reloads the stock pass inserts):
        u = ln(max(dist^2, 1e-8))   # max() on DVE kills fp32-negative noise
        v = exp(0.5*u + ln(256))    # = 256 * d
        w = exp(-v)                 # = exp(-256 d)
    row-sums on the Vector engine, then per 16-block group:
        t = 8 + 0.78125 * ln(sum + 1e-37)
        out = 1 / (1 + exp(t))      # = 1 - sigmoid(-t)
"""

import math

import ml_dtypes
import numpy as np

import concourse.bass as bass
import concourse.tile as tile
from concourse import bacc, mybir
from concourse.bass_utils import run_bass_kernel_spmd
from concourse.hw_specs import get_activation_tables
from concourse.masks import make_identity

SIZE = 512
N_SAMPLES = 32
N_STROKES = 16
NPTS = N_STROKES * N_SAMPLES  # 512
SHARP = float(N_SAMPLES) * 8.0  # 256
STROKE_WIDTH = 0.04
OUT_SCALE = 8.0 / STROKE_WIDTH  # 200

NCORES = 8
BLK = 32  # block side in pixels
NB = SIZE // BLK  # 16 blocks per image side
NBLOCKS = NB * NB  # 256
BLOCKS_PER_CORE = NBLOCKS // NCORES  # 32
PXB = BLK * BLK  # 1024 pixels per block
SUBT = PXB // 128  # 8 subtiles of 128 pixels
CUTOFF = 0.18  # candidate radius from block bbox
PADG = 16  # candidate count granularity
DUMMY = (3.0, 3.0)  # far-away pad point: exp(-256*d) == 0 in fp32
KROWS = 18  # bf16 limb-product rows in the matmul contraction
GRP = 8  # blocks per output group

f32 = mybir.dt.float32
bf16 = mybir.dt.bfloat16
np_bf16 = ml_dtypes.bfloat16
AF = mybir.ActivationFunctionType

_prog_cache: dict = {}


def _bezier_points(control_points: np.ndarray) -> np.ndarray:
    """[16,4,2] control points -> [512,2] float64 curve samples."""
    pts = np.clip(control_points.astype(np.float64), 0.0, 1.0)
    t = np.linspace(0.0, 1.0, N_SAMPLES)[None, :, None]
    mt = 1.0 - t
    p0, p1, p2, p3 = (pts[:, k : k + 1, :] for k in range(4))
    cur = mt**3 * p0 + 3 * mt**2 * t * p1 + 3 * mt * t**2 * p2 + t**3 * p3
    return cur.reshape(-1, 2)


def _split3(x: np.ndarray):
    """fp32-exact 3-way bf16 limb split (f64 in, 3x bf16 out)."""
    a = x.astype(np_bf16)
    r = x - a.astype(np.float64)
    b = r.astype(np_bf16)
    r = r - b.astype(np.float64)
    c = r.astype(np_bf16)
    return a, b, c


def _limb_rows(v1, v2, v3, w1, w2, w3, scale=1.0):
    """The 6 (stationary, moving) limb pairs covering v*w to ~2^-24:
    v1w1, v1w2, v2w1, v2w2, v1w3, v3w1."""
    sv = [v1, v1, v2, v2, v1, v3]
    sw = [w1, w2, w1, w2, w3, w1]
    if scale != 1.0:
        sv = [(s.astype(np.float64) * scale).astype(np_bf16) for s in sv]
    return sv, sw


def _batches(k_sched: tuple[int, ...]):
    """Group-aligned psum batches: (start_slot, nblk, K_pitch, fallback).
    A non-fallback batch packs nblk blocks' 8 subtile-results each into one
    4-bank psum tile at pitch K (bank = r%4, slot = r//4, r = j*8+st)."""
    out = []
    pos = 0
    n = len(k_sched)
    while pos < n:
        Kb = k_sched[pos]
        G = 512 // Kb
        if G < 2:
            out.append((pos, 1, Kb, True))
            pos += 1
        else:
            nblk = min(G // 2, n - pos, GRP - pos % GRP)
            out.append((pos, nblk, Kb, False))
            pos += nblk
    return tuple(out)


def _lift(k_sched: tuple[int, ...]):
    """Raise each slot's K to its batch pitch so every psum column is live."""
    k = list(k_sched)
    for start, nblk, Kb, fb in _batches(k_sched):
        for j in range(nblk):
            k[start + j] = Kb
    return tuple(k)


def _build_program(k_sched: tuple[int, ...]):
    """Build + compile the SPMD Bass program for a fixed per-slot candidate
    schedule. Returns (nc, mov_offsets)."""
    nslots = len(k_sched)
    ngroups = nslots // GRP
    mov_off = np.concatenate([[0], np.cumsum(k_sched)]).astype(int)
    mov_total = int(mov_off[-1])

    nc = bacc.Bacc(None, target_bir_lowering=False, num_swdge_queues=4)

    pix_d = nc.dram_tensor("pix", [KROWS, nslots * PXB], bf16, kind="ExternalInput")
    mov_d = nc.dram_tensor("mov", [KROWS, mov_total], bf16, kind="ExternalInput")
    out_d = nc.dram_tensor("out", [nslots * SUBT, 128], f32, kind="ExternalOutput")

    ln256 = math.log(SHARP)

    with tile.TileContext(nc) as tc:
        with (
            tc.tile_pool(name="io", bufs=1) as io,
            tc.tile_pool(name="work", bufs=3) as work,
            tc.tile_pool(name="acc", bufs=2) as acc,
            tc.tile_pool(name="fin", bufs=2) as fin,
            tc.tile_pool(name="psum", bufs=2, space="PSUM") as psum,
        ):
            # input DMAs first: anything else on gpsimd delays SWDGE kickoff
            mov_all = io.tile([KROWS, mov_total], bf16)
            nc.gpsimd.dma_start(mov_all[:], mov_d[:])
            pix_all = io.tile([KROWS, nslots * PXB], bf16)
            # graduated chunks so the first blocks start sooner
            csizes = [2, 2, 4, 4, 4, 8, 8]
            co = 0
            for cs in csizes:
                nc.gpsimd.dma_start(
                    pix_all[:, co * PXB : (co + cs) * PXB],
                    pix_d[:, co * PXB : (co + cs) * PXB],
                )
                co += cs
            ident = io.tile([128, 128], f32)
            make_identity(nc, ident)
            b_ln256 = io.tile([128, 1], f32)
            nc.vector.memset(b_ln256, ln256)
            b_tiny = io.tile([128, 1], f32)
            nc.vector.memset(b_tiny, 1e-37)
            b_eight = io.tile([128, 1], f32)
            nc.vector.memset(b_eight, STROKE_WIDTH * OUT_SCALE)

            def emit_final(g, sums):
                # t = 8 + 0.78125 * ln(sum + 1e-37); out = 1/(1 + exp(t))
                zt = fin.tile([128, GRP * SUBT], f32, tag="z")
                nc.scalar.activation(zt[:], sums[:], AF.Ln, bias=b_tiny[:])
                nc.scalar.activation(
                    zt[:], zt[:], AF.Exp, bias=b_eight[:], scale=OUT_SCALE / SHARP,
                )
                nc.vector.tensor_scalar_add(zt[:], zt[:], 1.0)
                nc.vector.reciprocal(zt[:], zt[:])
                # transpose so each output row is one subtile's 128 pixels
                ptt = psum.tile([128, 4, 512], f32, tag="ps")
                tview = ptt[: GRP * SUBT, 0, :128]
                nc.tensor.transpose(tview, zt[:], ident[:])
                ot = fin.tile([GRP * SUBT, 128], f32, tag="o")
                nc.vector.tensor_copy(ot[:], tview)
                nc.sync.dma_start(
                    out_d[g * GRP * SUBT : (g + 1) * GRP * SUBT, :], ot[:]
                )

            pending_final = None
            sums = None
            for start, nblk, Kb, fb in _batches(k_sched):
                g = start // GRP
                if start % GRP == 0:
                    if sums is not None:
                        pending_final = (g - 1, sums)
                    sums = acc.tile([128, GRP * SUBT], f32, tag="sums")
                R = nblk * SUBT
                ut = work.tile([128, 4096], f32, tag="u")
                if fb:
                    i = start
                    mov = mov_all[:, mov_off[i] : mov_off[i] + Kb]
                    for w in range(2):
                        pt = psum.tile([128, 4, 512], f32, tag="ps")
                        for ss in range(4):
                            st = w * 4 + ss
                            nc.tensor.matmul(
                                pt[:, ss, :Kb],
                                pix_all[:, i * PXB + st * 128 : i * PXB + (st + 1) * 128],
                                mov,
                                start=True,
                                stop=True,
                            )
                        # clamp fp32-negative dist^2; lay out r-major (r=w*4+b)
                        nc.vector.tensor_scalar_max(
                            ut[:, w * 4 * Kb : (w + 1) * 4 * Kb].rearrange(
                                "p (b k) -> p b k", k=Kb
                            ),
                            pt[:, :, :Kb],
                            1e-8,
                        )
                else:
                    pt = psum.tile([128, 4, 512], f32, tag="ps")
                    for j in range(nblk):
                        i = start + j
                        mov = mov_all[:, mov_off[i] : mov_off[i] + Kb]
                        for st in range(SUBT):
                            r = j * SUBT + st
                            nc.tensor.matmul(
                                pt[:, r % 4, (r // 4) * Kb : (r // 4 + 1) * Kb],
                                pix_all[:, i * PXB + st * 128 : i * PXB + (st + 1) * 128],
                                mov,
                                start=True,
                                stop=True,
                            )
                    S = R // 4
                    G = 512 // Kb
                    nc.vector.tensor_scalar_max(
                        ut[:, : R * Kb].rearrange("p (s b k) -> p b s k", b=4, k=Kb),
                        pt[:, :, : G * Kb].rearrange(
                            "p b (s k) -> p b s k", k=Kb
                        )[:, :, :S, :],
                        1e-8,
                    )
                # u = ln(dist^2); v = exp(0.5u + ln256) = 256d; w = exp(-v)
                nc.scalar.activation(ut[:, : R * Kb], ut[:, : R * Kb], AF.Ln)
                nc.scalar.activation(
                    ut[:, : R * Kb], ut[:, : R * Kb], AF.Exp,
                    bias=b_ln256[:], scale=0.5,
                )
                nc.scalar.activation(
                    ut[:, : R * Kb], ut[:, : R * Kb], AF.Exp, scale=-1.0
                )
                # per-pixel sums over each result's K candidates
                c0 = (start % GRP) * SUBT
                nc.vector.reduce_sum(
                    sums[:, c0 : c0 + R],
                    ut[:, : R * Kb].rearrange("p (r k) -> p r k", k=Kb),
                    axis=mybir.AxisListType.X,
                )
                if pending_final is not None:
                    emit_final(*pending_final)
                    pending_final = None
            emit_final(ngroups - 1, sums)

    nc.compile()

    # Dedup activation-table loads: every Ln/Exp in this kernel is served by
    # the one combined set, so keep the first load (retargeted to it) and
    # drop the rest.
    combined_id = None
    for idx, (name, funcs) in enumerate(get_activation_tables(nc.m.arch).items()):
        if {AF.Ln, AF.Exp} <= funcs:
            combined_id = idx
            break
    assert combined_id is not None, "no activation table set with both Ln and Exp"
    for blk in nc.m.functions[0].blocks:
        loads = [i for i in blk.instructions
                 if isinstance(i, mybir.InstLoadActFuncSet)]
        if not loads:
            continue
        loads[0].act_func_set_id = combined_id
        for l in loads[1:]:
            blk.instructions.remove(l)

    return nc, mov_off


def kernel(control_points: np.ndarray, pixel_grid: np.ndarray) -> np.ndarray:
    control_points = np.asarray(control_points, dtype=np.float32)
    pixel_grid = np.asarray(pixel_grid, dtype=np.float32)

    pts64 = _bezier_points(control_points)  # [512, 2] f64
    q64 = pts64.astype(np.float32).astype(np.float64)  # the fp32 values, exactly
    qn64 = q64[:, 0] ** 2 + q64[:, 1] ** 2

    # ---- block geometry from the actual pixel grid ----
    pg = pixel_grid.reshape(SIZE, SIZE, 2)
    # [NB, NB, BLK, BLK, 2] -> blocks (by, bx), local (lr, lc)
    pblk = pg.reshape(NB, BLK, NB, BLK, 2).transpose(0, 2, 1, 3, 4)
    pblk = np.ascontiguousarray(pblk).reshape(NBLOCKS, PXB, 2)
    bxmin = pblk[:, :, 0].min(1)
    bxmax = pblk[:, :, 0].max(1)
    bymin = pblk[:, :, 1].min(1)
    bymax = pblk[:, :, 1].max(1)

    # distance from each sample point to each block bbox
    dx = np.maximum(np.maximum(bxmin[:, None] - q64[None, :, 0],
                               q64[None, :, 0] - bxmax[:, None]), 0.0)
    dy = np.maximum(np.maximum(bymin[:, None] - q64[None, :, 1],
                               q64[None, :, 1] - bymax[:, None]), 0.0)
    # adaptive radius: every pixel in the block has a point within
    # dc_min + halfdiag, so points beyond that + 0.081 are invisible
    # (<= 512*exp(-256*0.081) ~ 5e-7 relative) wherever the output is not 1.0f
    ccx = 0.5 * (bxmin + bxmax)
    ccy = 0.5 * (bymin + bymax)
    dc_min = np.sqrt((ccx[:, None] - q64[None, :, 0]) ** 2
                     + (ccy[:, None] - q64[None, :, 1]) ** 2).min(1)
    r_b = np.minimum(CUTOFF, dc_min + 0.125)
    cand = dx * dx + dy * dy < (r_b[:, None] + 1e-3) ** 2  # [NBLOCKS, 512]
    kcnt = cand.sum(1)
    kpad = np.maximum(((kcnt + PADG - 1) // PADG) * PADG, PADG).astype(int)

    # ---- LPT assignment: exactly BLOCKS_PER_CORE blocks per core ----
    order = np.argsort(-kpad, kind="stable")
    loads = np.zeros(NCORES)
    counts = np.zeros(NCORES, dtype=int)
    assign = np.zeros(NBLOCKS, dtype=int)
    for b in order:
        elig = np.flatnonzero(counts < BLOCKS_PER_CORE)
        c = elig[np.argmin(loads[elig])]
        assign[b] = c
        loads[c] += kpad[b]
        counts[c] += 1

    # per-core slots sorted by descending kpad; shared schedule = slotwise max
    core_blocks = []
    for c in range(NCORES):
        blks = np.flatnonzero(assign == c)
        blks = blks[np.argsort(-kpad[blks], kind="stable")]
        core_blocks.append(blks)
    core_blocks = np.stack(core_blocks)  # [8, 32]
    k_sched = tuple(int(kpad[core_blocks[:, i]].max()) for i in range(BLOCKS_PER_CORE))
    k_sched = _lift(k_sched)

    if k_sched not in _prog_cache:
        _prog_cache.clear()
        _prog_cache[k_sched] = _build_program(k_sched)
    nc, mov_off = _prog_cache[k_sched]
    mov_total = int(mov_off[-1])

    # ---- moving-side limb rows (shared tables, gathered per block) ----
    q1x, q2x, q3x = _split3(q64[:, 0])
    q1y, q2y, q3y = _split3(q64[:, 1])
    qn1, qn2, qn3 = _split3(qn64)
    ones = np.ones(NPTS, dtype=np_bf16)
    mov_rows_all = np.stack(
        [qn1, qn2, qn3,
         q1x, q2x, q1x, q2x, q3x, q1x,
         q1y, q2y, q1y, q2y, q3y, q1y,
         ones, ones, ones]
    )  # [18, 512] bf16

    dum = np.float64(DUMMY[0])
    d1, d2, d3 = _split3(np.array([dum]))
    dn1, dn2, dn3 = _split3(np.array([2 * dum * dum]))
    mov_dummy = np.array(
        [dn1[0], dn2[0], dn3[0],
         d1[0], d2[0], d1[0], d2[0], d3[0], d1[0],
         d1[0], d2[0], d1[0], d2[0], d3[0], d1[0],
         1.0, 1.0, 1.0], dtype=np_bf16)

    # ---- per-core input arrays ----
    in_maps = []
    for c in range(NCORES):
        pix = np.empty((KROWS, BLOCKS_PER_CORE * PXB), dtype=np_bf16)
        mov = np.empty((KROWS, mov_total), dtype=np_bf16)
        mov[:] = mov_dummy[:, None]
        for i, b in enumerate(core_blocks[c]):
            px = pblk[b].astype(np.float64)  # [1024, 2]
            sl = slice(i * PXB, (i + 1) * PXB)
            p1x, p2x, p3x = _split3(px[:, 0])
            p1y, p2y, p3y = _split3(px[:, 1])
            pn1, pn2, pn3 = _split3(px[:, 0] ** 2 + px[:, 1] ** 2)
            svx, _ = _limb_rows(p1x, p2x, p3x, None, None, None, scale=-2.0)
            svy, _ = _limb_rows(p1y, p2y, p3y, None, None, None, scale=-2.0)
            po = np.ones(PXB, dtype=np_bf16)
            pix[:, sl] = np.stack([po, po, po] + svx + svy + [pn1, pn2, pn3])
            idx = np.flatnonzero(cand[b])
            o = int(mov_off[i])
            mov[:, o : o + len(idx)] = mov_rows_all[:, idx]
        in_maps.append({"pix": pix, "mov": mov})

    global _last_in_maps
    _last_in_maps = in_maps
    res = run_bass_kernel_spmd(nc, in_maps, core_ids=list(range(NCORES)))

    # ---- unshard: scatter block results back into the image ----
    img = np.empty(SIZE * SIZE, dtype=np.float32)
    by, bx = np.meshgrid(np.arange(NB), np.arange(NB), indexing="ij")
    lr, lc = np.meshgrid(np.arange(BLK), np.arange(BLK), indexing="ij")
    flat = ((by.reshape(-1, 1) * BLK + lr.reshape(-1)[None, :]) * SIZE
            + bx.reshape(-1, 1) * BLK + lc.reshape(-1)[None, :])  # [NBLOCKS, PXB]
    for c in range(NCORES):
        o = res.results[c]["out"].reshape(BLOCKS_PER_CORE, PXB)
        for i, b in enumerate(core_blocks[c]):
            img[flat[b]] = o[i]
    return img.reshape(1, SIZE, SIZE)



# revision 5
# speedup vs baseline: 2.0151x; 2.0151x over previous
"""Trainium2 Bass kernel for nn_BezierGlyph (retrieval_knn).

Math (matching the jax reference):
  pts  = cubic-bezier samples of clip(control_points, 0, 1)   # [512, 2]
  d_ij = |pixel_i - pts_j|
  m_i  = -logsumexp(-256 * d_i:) / 256                        # softmin
  out  = 1 - sigmoid((0.04 - m) * 200)                        # (1, 512, 512)

Strategy (sharding_hint: shard pixels, replicate points):
  * 512x512 pixels in 16x16 blocks (1024). Blocks with no sample point
    within min(0.151, dc_min + 0.070) of their bbox output exactly 1.0f
    and are skipped entirely (host writes the 1.0s). The ~700 live
    blocks are dealt round-robin (sorted by candidate count) over the
    8 cores; the shared SPMD schedule is the slot-wise max.
  * Coordinates are re-centered per block: d^2 = |q-c|^2 + |p-c|^2
    - 2(p-c).(q-c) via one PE matmul with an 11-row bf16 limb
    contraction (2-limb splits suffice at these magnitudes; worst-case
    |noise| ~3e-7, clamped by the sqrt bias below).
  * Two activation passes instead of three:
        v = sqrt(d^2 + 6e-7)        # Sqrt table
        w = exp(-256 * v)           # natural_log_exp table
    The kernel is split into a sqrt phase and an exp phase separated by
    a scheduler-only fence (tc.no_sync_barrier) so the final ACT stream
    needs exactly two table loads (a post-compile pass dedups the
    per-instruction reloads the stock pass inserts).
  * The Sqrt pass reads PSUM directly (no DVE clamp/copy pass); the Exp
    pass runs in place on SBUF; DVE only does the per-result row sums.
  * Per 64-slot group: t = 8 + 0.78125*ln(sum + 1e-37);
    out = 1/(1 + exp(t)), DMA'd untransposed as [128, 2*nslots]
    (host transposes).
"""

import math

import ml_dtypes
import numpy as np

import concourse.bass as bass
import concourse.tile as tile
from concourse import bacc, mybir
from concourse.bass_utils import run_bass_kernel_spmd
from concourse.hw_specs import get_activation_tables

SIZE = 512
N_SAMPLES = 32
N_STROKES = 16
NPTS = N_STROKES * N_SAMPLES  # 512
SHARP = float(N_SAMPLES) * 8.0  # 256
STROKE_WIDTH = 0.04
OUT_SCALE = 8.0 / STROKE_WIDTH  # 200

NCORES = 8
BLK = 16  # block side in pixels
NB = SIZE // BLK  # 32
NBLOCKS = NB * NB  # 1024
PXB = BLK * BLK  # 256 pixels per block
SUBT = PXB // 128  # 2 subtiles of 128 pixels
HALFDIAG = BLK / SIZE * math.sqrt(2.0) / 2.0  # 0.0221
DELTA = 0.048  # points beyond dmin+DELTA are invisible (<=1e-3 out err)
CUTOFF = 0.103 + DELTA  # 0.103 = boring-pixel min_dist bound
PADG = 4  # candidate count granularity
KROWS = 11  # bf16 limb-product rows in the matmul contraction
GRP = 64  # slots per output group
SQ_BIAS = 6e-7  # clamps fp noise in d^2 (|noise| <~ 3e-7)

f32 = mybir.dt.float32
bf16 = mybir.dt.bfloat16
np_bf16 = ml_dtypes.bfloat16
AF = mybir.ActivationFunctionType

_prog_cache: dict = {}
_last_in_maps = None


def _bezier_points(control_points: np.ndarray) -> np.ndarray:
    """[16,4,2] control points -> [512,2] float64 curve samples."""
    pts = np.clip(control_points.astype(np.float64), 0.0, 1.0)
    t = np.linspace(0.0, 1.0, N_SAMPLES)[None, :, None]
    mt = 1.0 - t
    p0, p1, p2, p3 = (pts[:, k : k + 1, :] for k in range(4))
    cur = mt**3 * p0 + 3 * mt**2 * t * p1 + 3 * mt * t**2 * p2 + t**3 * p3
    return cur.reshape(-1, 2)


def _split2(x: np.ndarray):
    """2-way bf16 limb split (f64 in, 2x bf16 out; remainder ~2^-16 rel)."""
    a = x.astype(np_bf16)
    b = (x - a.astype(np.float64)).astype(np_bf16)
    return a, b


def _split3(x: np.ndarray):
    a = x.astype(np_bf16)
    r = x - a.astype(np.float64)
    b = r.astype(np_bf16)
    c = (r - b.astype(np.float64)).astype(np_bf16)
    return a, b, c


def _batches(k_sched: tuple[int, ...]):
    """Uniform-pitch psum batches: (start_slot, nslots, pitch). A batch's
    2*nslots results pack into one 4-bank psum tile at pitch P (bank
    r//rpb, offset (r%rpb)*P, rpb = 512//P). Batches stay within one
    output group and stop extending when the pitch-lift would exceed
    4/3x."""
    out = []
    pos = 0
    n = len(k_sched)
    while pos < n:
        P = k_sched[pos]
        rpb = 512 // P
        lim = min((4 * rpb) // SUBT, n - pos, GRP - pos % GRP)
        j = 1
        while j < lim and 4 * k_sched[pos + j] >= 3 * P:
            j += 1
        out.append((pos, j, P))
        pos += j
    return tuple(out)


def _lift(k_sched: tuple[int, ...]):
    """Raise each slot's K to its batch pitch."""
    k = list(k_sched)
    for start, nb, P in _batches(k_sched):
        for j in range(nb):
            k[start + j] = P
    return tuple(k)


def _build_program(k_sched: tuple[int, ...]):
    """Build + compile the SPMD Bass program for a fixed per-slot candidate
    schedule. Returns (nc, mov_off)."""
    nslots = len(k_sched)
    batches = _batches(k_sched)
    lifted = _lift(k_sched)
    mov_off = np.concatenate([[0], np.cumsum(lifted)]).astype(int)
    mov_total = int(mov_off[-1])
    ngroups = (nslots + GRP - 1) // GRP

    # vt column offset per batch (packed, no gaps)
    voff = []
    tot = 0
    for start, nb, P in batches:
        voff.append(tot)
        tot += SUBT * nb * P

    nc = bacc.Bacc(None, target_bir_lowering=False, num_swdge_queues=4)

    pix_d = nc.dram_tensor("pix", [KROWS, nslots * PXB], bf16, kind="ExternalInput")
    mov_d = nc.dram_tensor("mov", [KROWS, mov_total], bf16, kind="ExternalInput")
    out_d = nc.dram_tensor("out", [128, nslots * SUBT], f32, kind="ExternalOutput")

    with tile.TileContext(nc) as tc:
        with (
            tc.tile_pool(name="io", bufs=1) as io,
            tc.tile_pool(name="vtp", bufs=1) as vtp,
            tc.tile_pool(name="acc", bufs=2) as acc,
            tc.tile_pool(name="fin", bufs=2) as fin,
            tc.tile_pool(name="psum", bufs=2, space="PSUM") as psum,
        ):
            pix_all = io.tile([KROWS, nslots * PXB], bf16)
            mov_all = io.tile([KROWS, mov_total], bf16)
            # graduated input chunks at batch boundaries; first from the
            # sync engine (HWDGE, cheapest kickoff), bulk via gpsimd
            bnds = [batches[i][0] for i in (1, 2, 4) if i < len(batches)]
            bnds = sorted(set(b for b in bnds if 0 < b < nslots)) + [nslots]
            engs = [nc.sync, nc.sync, nc.gpsimd, nc.gpsimd]
            s0 = 0
            for ci, s1 in enumerate(bnds):
                eng = engs[min(ci, 3)]
                eng.dma_start(
                    pix_all[:, s0 * PXB : s1 * PXB], pix_d[:, s0 * PXB : s1 * PXB]
                )
                m0, m1 = int(mov_off[s0]), int(mov_off[s1])
                if m1 > m0:
                    eng.dma_start(mov_all[:, m0:m1], mov_d[:, m0:m1])
                s0 = s1
            b_sqb = io.tile([128, 1], f32)
            nc.vector.memset(b_sqb, SQ_BIAS)
            b_tiny = io.tile([128, 1], f32)
            nc.vector.memset(b_tiny, 1e-37)
            b_eight = io.tile([128, 1], f32)
            nc.vector.memset(b_eight, STROKE_WIDTH * OUT_SCALE)

            vt = vtp.tile([128, tot], f32)

            # ---- phase A: matmuls + sqrt (Sqrt table) ----
            for bi, (start, nb, P) in enumerate(batches):
                rpb = 512 // P
                nr = SUBT * nb
                pt = psum.tile([128, 4, 512], f32, tag="ps")
                for j in range(nb):
                    i = start + j
                    mov = mov_all[:, mov_off[i] : mov_off[i] + P]
                    for st in range(SUBT):
                        r = SUBT * j + st
                        nc.tensor.matmul(
                            pt[:, r // rpb, (r % rpb) * P : (r % rpb + 1) * P],
                            pix_all[:, i * PXB + st * 128 : i * PXB + (st + 1) * 128],
                            mov,
                            start=True,
                            stop=True,
                        )
                nbf, rem = nr // rpb, nr % rpb
                vo = voff[bi]
                if nbf:
                    nc.scalar.activation(
                        vt[:, vo : vo + nbf * rpb * P].rearrange(
                            "p (b c) -> p b c", c=rpb * P
                        ),
                        pt[:, :nbf, : rpb * P],
                        AF.Sqrt,
                        bias=b_sqb[:],
                    )
                if rem:
                    nc.scalar.activation(
                        vt[:, vo + nbf * rpb * P : vo + nr * P],
                        pt[:, nbf, : rem * P],
                        AF.Sqrt,
                        bias=b_sqb[:],
                    )

            # ACT stream fence: all Sqrts schedule before any Exp/Ln so
            # exactly two table loads survive. Scheduler-only, no sems.
            tc.no_sync_barrier()

            # ---- phase B: exp + row sums + per-group finalization ----
            sums = None
            for bi, (start, nb, P) in enumerate(batches):
                g = start // GRP
                if start % GRP == 0:
                    sums = acc.tile([128, GRP * SUBT], f32, tag="sums")
                nr = SUBT * nb
                vo = voff[bi]
                nc.scalar.activation(
                    vt[:, vo : vo + nr * P],
                    vt[:, vo : vo + nr * P],
                    AF.Exp,
                    scale=-SHARP,
                )
                c0 = (start % GRP) * SUBT
                nc.vector.reduce_sum(
                    sums[:, c0 : c0 + nr],
                    vt[:, vo : vo + nr * P].rearrange("p (r k) -> p r k", k=P),
                    axis=mybir.AxisListType.X,
                )
                last_of_group = (
                    bi + 1 == len(batches) or batches[bi + 1][0] // GRP != g
                )
                if last_of_group:
                    n = min(GRP, nslots - g * GRP) * SUBT
                    zt = fin.tile([128, GRP * SUBT], f32, tag="z")
                    nc.scalar.activation(
                        zt[:, :n], sums[:, :n], AF.Ln, bias=b_tiny[:]
                    )
                    nc.scalar.activation(
                        zt[:, :n], zt[:, :n], AF.Exp, bias=b_eight[:],
                        scale=OUT_SCALE / SHARP,
                    )
                    nc.vector.tensor_scalar_add(zt[:, :n], zt[:, :n], 1.0)
                    nc.vector.reciprocal(zt[:, :n], zt[:, :n])
                    c = g * GRP * SUBT
                    nc.sync.dma_start(out_d[:, c : c + n], zt[:, :n])

    nc.compile()

    # Keep one table load per phase: retarget the first load before a
    # Sqrt to the sqrt set, the first before an Exp/Ln to the combined
    # natural-log/exp set, and drop the redundant reloads in between.
    tables = list(get_activation_tables(nc.m.arch).items())
    sqrt_id = next(i for i, (_, fs) in enumerate(tables) if AF.Sqrt in fs)
    nl_id = next(i for i, (_, fs) in enumerate(tables) if {AF.Ln, AF.Exp} <= fs)
    for blk in nc.m.functions[0].blocks:
        cur = None
        pending = []
        for ins in list(blk.instructions):
            if isinstance(ins, mybir.InstLoadActFuncSet):
                pending.append(ins)
            elif isinstance(ins, mybir.InstActivation):
                need = sqrt_id if ins.func == AF.Sqrt else nl_id
                if pending:
                    if need != cur:
                        pending[0].act_func_set_id = need
                        for l in pending[1:]:
                            blk.instructions.remove(l)
                        cur = need
                    else:
                        for l in pending:
                            blk.instructions.remove(l)
                    pending = []
                else:
                    assert cur == need, "activation without table load"
        for l in pending:
            blk.instructions.remove(l)

    return nc, mov_off


def kernel(control_points: np.ndarray, pixel_grid: np.ndarray) -> np.ndarray:
    control_points = np.asarray(control_points, dtype=np.float32)
    pixel_grid = np.asarray(pixel_grid, dtype=np.float32)

    pts64 = _bezier_points(control_points)
    q64 = pts64.astype(np.float32).astype(np.float64)  # the fp32 values, exactly

    # ---- block geometry from the actual pixel grid ----
    pg = pixel_grid.reshape(SIZE, SIZE, 2)
    pblk = pg.reshape(NB, BLK, NB, BLK, 2).transpose(0, 2, 1, 3, 4)
    pblk = np.ascontiguousarray(pblk).reshape(NBLOCKS, PXB, 2).astype(np.float64)
    bxmin = pblk[:, :, 0].min(1)
    bxmax = pblk[:, :, 0].max(1)
    bymin = pblk[:, :, 1].min(1)
    bymax = pblk[:, :, 1].max(1)
    ccx = 0.5 * (bxmin + bxmax)
    ccy = 0.5 * (bymin + bymax)

    dx = np.maximum(np.maximum(bxmin[:, None] - q64[None, :, 0],
                               q64[None, :, 0] - bxmax[:, None]), 0.0)
    dy = np.maximum(np.maximum(bymin[:, None] - q64[None, :, 1],
                               q64[None, :, 1] - bymax[:, None]), 0.0)
    dc_min = np.sqrt((ccx[:, None] - q64[None, :, 0]) ** 2
                     + (ccy[:, None] - q64[None, :, 1]) ** 2).min(1)
    r_b = np.minimum(CUTOFF, dc_min + HALFDIAG + DELTA)
    cand = dx * dx + dy * dy < (r_b[:, None] + 5e-4) ** 2  # [NBLOCKS, 512]
    kcnt = cand.sum(1)
    nonempty = np.flatnonzero(kcnt > 0)
    kpad = (((kcnt + PADG - 1) // PADG) * PADG).astype(int)

    img = np.ones(SIZE * SIZE, dtype=np.float32)
    if len(nonempty) == 0:
        return img.reshape(1, SIZE, SIZE)

    # ---- deal blocks (sorted desc by padded count) round-robin ----
    order = nonempty[np.argsort(-kpad[nonempty], kind="stable")]
    nslots = (len(order) + NCORES - 1) // NCORES
    core_blocks = [order[c::NCORES] for c in range(NCORES)]
    k_sched = tuple(int(kpad[order[NCORES * i]]) for i in range(nslots))

    if k_sched not in _prog_cache:
        _prog_cache.clear()
        _prog_cache[k_sched] = _build_program(k_sched)
    nc, mov_off = _prog_cache[k_sched]
    mov_total = int(mov_off[-1])

    # ---- shared per-block pix rows (block-centered, bf16 limbs) ----
    # rows: [1,1,1, -2x1,-2x1,-2x2, -2y1,-2y1,-2y2, pn1,pn2] pairing mov
    #       [qn1,qn2,qn3, qx1,qx2,qx1, qy1,qy2,qy1, 1,1]
    nb_ne = len(order)
    c_ne = np.stack([ccx[order], ccy[order]], axis=1)  # [nb_ne, 2]
    prel = pblk[order] - c_ne[:, None, :]  # [nb_ne, 256, 2]
    x1, x2 = _split2(prel[:, :, 0])
    y1, y2 = _split2(prel[:, :, 1])
    pn1, pn2 = _split2(prel[:, :, 0] ** 2 + prel[:, :, 1] ** 2)
    ones = np.ones_like(x1)
    pix_ne = np.stack(
        [ones, ones, ones,
         -2.0 * x1, -2.0 * x1, -2.0 * x2,
         -2.0 * y1, -2.0 * y1, -2.0 * y2,
         pn1, pn2], axis=1,
    ).astype(np_bf16)  # [nb_ne, 11, 256]

    # far dummy candidate (relative coords): qrel=(4,4) -> d >= 5.6
    dn1, dn2, dn3 = _split3(np.array([32.0]))
    d1, d2 = _split2(np.array([4.0]))
    mov_dummy = np.array(
        [dn1[0], dn2[0], dn3[0],
         d1[0], d2[0], d1[0],
         d1[0], d2[0], d1[0],
         1.0, 1.0], dtype=np_bf16)

    # ---- per-core input arrays ----
    in_maps = []
    for c in range(NCORES):
        blks = core_blocks[c]
        pix = np.zeros((KROWS, nslots * PXB), dtype=np_bf16)
        mov = np.empty((KROWS, mov_total), dtype=np_bf16)
        mov[:] = mov_dummy[:, None]
        for i, b in enumerate(blks):
            gi = NCORES * i + c  # index into `order`
            pix[:, i * PXB : (i + 1) * PXB] = pix_ne[gi]
            idx = np.flatnonzero(cand[b])
            qrel = q64[idx] - c_ne[gi][None, :]  # [K, 2]
            qx1, qx2 = _split2(qrel[:, 0])
            qy1, qy2 = _split2(qrel[:, 1])
            qn1, qn2, qn3 = _split3(qrel[:, 0] ** 2 + qrel[:, 1] ** 2)
            o = int(mov_off[i])
            mov[:, o : o + len(idx)] = np.stack(
                [qn1, qn2, qn3, qx1, qx2, qx1, qy1, qy2, qy1,
                 np.ones_like(qx1), np.ones_like(qx1)])
        in_maps.append({"pix": pix, "mov": mov})

    global _last_in_maps
    _last_in_maps = in_maps
    res = run_bass_kernel_spmd(nc, in_maps, core_ids=list(range(NCORES)))

    # ---- unshard: scatter block results back into the image ----
    by, bx = np.meshgrid(np.arange(NB), np.arange(NB), indexing="ij")
    lr, lc = np.meshgrid(np.arange(BLK), np.arange(BLK), indexing="ij")
    flat = ((by.reshape(-1, 1) * BLK + lr.reshape(-1)[None, :]) * SIZE
            + bx.reshape(-1, 1) * BLK + lc.reshape(-1)[None, :])  # [NBLOCKS, PXB]
    for c in range(NCORES):
        blks = core_blocks[c]
        o = res.results[c]["out"].T.reshape(nslots, PXB)  # [nslots, 256]
        img[flat[blks]] = o[: len(blks)]
    return img.reshape(1, SIZE, SIZE)


# revision 6
# speedup vs baseline: 2.1659x; 1.0748x over previous
"""Trainium2 Bass kernel for nn_BezierGlyph (retrieval_knn).

Math (matching the jax reference):
  pts  = cubic-bezier samples of clip(control_points, 0, 1)   # [512, 2]
  d_ij = |pixel_i - pts_j|
  m_i  = -logsumexp(-256 * d_i:) / 256                        # softmin
  out  = 1 - sigmoid((0.04 - m) * 200)                        # (1, 512, 512)

Strategy (sharding_hint: shard pixels, replicate points):
  * 512x512 pixels in 16x16 blocks (1024). Blocks with no sample point
    within min(0.151, dc_min + 0.070) of their bbox output exactly 1.0f
    and are skipped entirely (host writes the 1.0s). The ~700 live
    blocks are dealt round-robin (sorted by candidate count) over the
    8 cores; the shared SPMD schedule is the slot-wise max.
  * Coordinates are re-centered per block: d^2 = |q-c|^2 + |p-c|^2
    - 2(p-c).(q-c) via an 11-row bf16 limb contraction (2-limb splits
    suffice at these magnitudes; worst-case |noise| ~3e-7, clamped by
    the sqrt bias below). Up to 4 results (128-pixel subtiles) share
    one matmul: their 11-row groups stack on the contraction axis and
    the moving operand is block-diagonal — per-matmul overhead (~60ns)
    dominates at these sizes, so fewer/fatter matmuls win.
  * Two activation passes instead of three:
        v = sqrt(d^2 + 6e-7)        # Sqrt table
        w = exp(-256 * v)           # natural_log_exp table
    The kernel is split into a sqrt phase and an exp phase separated by
    a scheduler-only fence (tc.no_sync_barrier) so the final ACT stream
    needs exactly two table loads (a post-compile pass dedups the
    per-instruction reloads the stock pass inserts).
  * The Sqrt pass reads PSUM directly (no DVE clamp/copy pass); the Exp
    pass runs in place on SBUF; DVE only does the per-result row sums.
  * Inputs arrive as ONE dram tensor in three graduated chunks (first
    two batches, next three, rest) so the first matmul starts early and
    the bulk streams behind it.
  * Per 64-slot group: t = 8 + 0.78125*ln(sum + 1e-37);
    out = 1/(1 + exp(t)), DMA'd untransposed as [128, 2*nslots]
    (host transposes).
"""

import math

import ml_dtypes
import numpy as np

import concourse.bass as bass
import concourse.tile as tile
from concourse import bacc, mybir
from concourse.bass_utils import run_bass_kernel_spmd
from concourse.hw_specs import get_activation_tables

SIZE = 512
N_SAMPLES = 32
N_STROKES = 16
NPTS = N_STROKES * N_SAMPLES  # 512
SHARP = float(N_SAMPLES) * 8.0  # 256
STROKE_WIDTH = 0.04
OUT_SCALE = 8.0 / STROKE_WIDTH  # 200

NCORES = 8
BLK = 16  # block side in pixels
NB = SIZE // BLK  # 32
NBLOCKS = NB * NB  # 1024
PXB = BLK * BLK  # 256 pixels per block
SUBT = PXB // 128  # 2 subtiles of 128 pixels
HALFDIAG = BLK / SIZE * math.sqrt(2.0) / 2.0  # 0.0221
DELTA = 0.048  # points beyond dmin+DELTA are invisible (<=1e-3 out err)
CUTOFF = 0.103 + DELTA  # 0.103 = boring-pixel min_dist bound
PADG = 4  # candidate count granularity
KROWS = 11  # bf16 limb-product rows per result in the contraction
MMPACK = 4  # max results stacked per matmul (44 contraction rows)
GRP = 64  # slots per output group
SQ_BIAS = 6e-7  # clamps fp noise in d^2 (|noise| <~ 3e-7)

f32 = mybir.dt.float32
bf16 = mybir.dt.bfloat16
np_bf16 = ml_dtypes.bfloat16
AF = mybir.ActivationFunctionType

_prog_cache: dict = {}
_last_in_maps = None


def _bezier_points(control_points: np.ndarray) -> np.ndarray:
    """[16,4,2] control points -> [512,2] float64 curve samples."""
    pts = np.clip(control_points.astype(np.float64), 0.0, 1.0)
    t = np.linspace(0.0, 1.0, N_SAMPLES)[None, :, None]
    mt = 1.0 - t
    p0, p1, p2, p3 = (pts[:, k : k + 1, :] for k in range(4))
    cur = mt**3 * p0 + 3 * mt**2 * t * p1 + 3 * mt * t**2 * p2 + t**3 * p3
    return cur.reshape(-1, 2)


def _split2(x: np.ndarray):
    """2-way bf16 limb split (f64 in, 2x bf16 out; remainder ~2^-16 rel)."""
    a = x.astype(np_bf16)
    b = (x - a.astype(np.float64)).astype(np_bf16)
    return a, b


def _split3(x: np.ndarray):
    a = x.astype(np_bf16)
    r = x - a.astype(np.float64)
    b = r.astype(np_bf16)
    c = (r - b.astype(np.float64)).astype(np_bf16)
    return a, b, c


def _plan(k_sched: tuple[int, ...]):
    """Shared host/builder plan for a fixed per-slot candidate schedule.

    Batches: uniform-pitch psum tiles — a batch's 2*nslots results pack
    into one 4-bank tile at pitch P (bank r//rpb, offset (r%rpb)*P,
    rpb = 512//P); batches stay within one output group and stop
    extending when the pitch-lift would exceed 4/3x.

    Matmul groups: up to MMPACK consecutive same-bank results stack
    into one matmul (11 contraction rows each, block-diagonal moving
    operand).

    Input columns: one dram tensor, ordered chunk-by-chunk with each
    chunk's stationary (pix) columns first, then its moving (mov)
    columns, so each chunk is one contiguous DMA.
    """
    nslots = len(k_sched)
    batches = []
    pos = 0
    while pos < nslots:
        P = k_sched[pos]
        rpb = 512 // P
        lim = min((4 * rpb) // SUBT, nslots - pos, GRP - pos % GRP)
        j = 1
        while j < lim and 4 * k_sched[pos + j] >= 3 * P:
            j += 1
        batches.append((pos, j, P))
        pos += j

    lifted = list(k_sched)
    for start, nb, P in batches:
        for j in range(nb):
            lifted[start + j] = P

    # chunk id per batch: 0 = first two, 1 = next three, 2 = rest
    def chunk_of(bi):
        return 0 if bi < 2 else (1 if bi < 5 else 2)

    groups = []  # (bi, bank, r0, F, P)
    for bi, (start, nb, P) in enumerate(batches):
        rpb = 512 // P
        nr = SUBT * nb
        F = min(rpb, MMPACK)
        for b0 in range(0, nr, rpb):
            bend = min(b0 + rpb, nr)
            for r0 in range(b0, bend, F):
                groups.append((bi, b0 // rpb, r0, min(F, bend - r0), P))

    # vt column offset per batch (packed, no gaps)
    voff = []
    tot = 0
    for start, nb, P in batches:
        voff.append(tot)
        tot += SUBT * nb * P

    # input column layout
    pixcol = [0] * len(groups)
    movcol = [0] * len(groups)
    chunk_bounds = []
    off = 0
    for c in range(3):
        gs = [gi for gi, g in enumerate(groups) if chunk_of(g[0]) == c]
        for gi in gs:
            pixcol[gi] = off
            off += 128
        for gi in gs:
            movcol[gi] = off
            off += groups[gi][3] * groups[gi][4]
        chunk_bounds.append(off)
    totc = off

    return {
        "nslots": nslots,
        "batches": batches,
        "lifted": lifted,
        "groups": groups,
        "voff": voff,
        "vtot": tot,
        "pixcol": pixcol,
        "movcol": movcol,
        "chunk_bounds": chunk_bounds,
        "totc": totc,
        "ngroups": (nslots + GRP - 1) // GRP,
    }


def _build_program(k_sched: tuple[int, ...]):
    plan = _plan(k_sched)
    nslots = plan["nslots"]
    batches = plan["batches"]
    groups = plan["groups"]
    voff = plan["voff"]
    pixcol = plan["pixcol"]
    movcol = plan["movcol"]
    totc = plan["totc"]
    PROWS = KROWS * MMPACK

    nc = bacc.Bacc(None, target_bir_lowering=False, num_swdge_queues=4)

    inp_d = nc.dram_tensor("inp", [PROWS, totc], bf16, kind="ExternalInput")
    out_d = nc.dram_tensor("out", [128, nslots * SUBT], f32, kind="ExternalOutput")

    with tile.TileContext(nc) as tc:
        with (
            tc.tile_pool(name="io", bufs=1) as io,
            tc.tile_pool(name="vtp", bufs=1) as vtp,
            tc.tile_pool(name="acc", bufs=2) as acc,
            tc.tile_pool(name="fin", bufs=2) as fin,
            tc.tile_pool(name="psum", bufs=2, space="PSUM") as psum,
        ):
            inp_all = io.tile([PROWS, totc], bf16)
            engs = [nc.sync, nc.gpsimd, nc.gpsimd]
            c0 = 0
            for ci, c1 in enumerate(plan["chunk_bounds"]):
                if c1 > c0:
                    engs[ci].dma_start(inp_all[:, c0:c1], inp_d[:, c0:c1])
                c0 = c1
            b_sqb = io.tile([128, 1], f32)
            nc.vector.memset(b_sqb, SQ_BIAS)
            b_tiny = io.tile([128, 1], f32)
            nc.vector.memset(b_tiny, 1e-37)
            b_eight = io.tile([128, 1], f32)
            nc.vector.memset(b_eight, STROKE_WIDTH * OUT_SCALE)

            vt = vtp.tile([128, plan["vtot"]], f32)

            # ---- phase A: matmuls + sqrt (Sqrt table) ----
            gi = 0
            for bi, (start, nb, P) in enumerate(batches):
                rpb = 512 // P
                nr = SUBT * nb
                pt = psum.tile([128, 4, 512], f32, tag="ps")
                while gi < len(groups) and groups[gi][0] == bi:
                    _, bank, r0, F, _ = groups[gi]
                    o = (r0 % rpb) * P
                    nc.tensor.matmul(
                        pt[:, bank, o : o + F * P],
                        inp_all[: KROWS * F, pixcol[gi] : pixcol[gi] + 128],
                        inp_all[: KROWS * F, movcol[gi] : movcol[gi] + F * P],
                        start=True,
                        stop=True,
                    )
                    gi += 1
                nbf, rem = nr // rpb, nr % rpb
                vo = voff[bi]
                if nbf:
                    nc.scalar.activation(
                        vt[:, vo : vo + nbf * rpb * P].rearrange(
                            "p (b c) -> p b c", c=rpb * P
                        ),
                        pt[:, :nbf, : rpb * P],
                        AF.Sqrt,
                        bias=b_sqb[:],
                    )
                if rem:
                    nc.scalar.activation(
                        vt[:, vo + nbf * rpb * P : vo + nr * P],
                        pt[:, nbf, : rem * P],
                        AF.Sqrt,
                        bias=b_sqb[:],
                    )

            # ACT stream fence: all Sqrts schedule before any Exp/Ln so
            # exactly two table loads survive. Scheduler-only, no sems.
            tc.no_sync_barrier()

            # ---- phase B: exp + row sums + per-group finalization ----
            sums = None
            for bi, (start, nb, P) in enumerate(batches):
                g = start // GRP
                if start % GRP == 0:
                    sums = acc.tile([128, GRP * SUBT], f32, tag="sums")
                nr = SUBT * nb
                vo = voff[bi]
                nc.scalar.activation(
                    vt[:, vo : vo + nr * P],
                    vt[:, vo : vo + nr * P],
                    AF.Exp,
                    scale=-SHARP,
                )
                cs = (start % GRP) * SUBT
                nc.vector.reduce_sum(
                    sums[:, cs : cs + nr],
                    vt[:, vo : vo + nr * P].rearrange("p (r k) -> p r k", k=P),
                    axis=mybir.AxisListType.X,
                )
                last_of_group = (
                    bi + 1 == len(batches) or batches[bi + 1][0] // GRP != g
                )
                if last_of_group:
                    n = min(GRP, nslots - g * GRP) * SUBT
                    zt = fin.tile([128, GRP * SUBT], f32, tag="z")
                    nc.scalar.activation(
                        zt[:, :n], sums[:, :n], AF.Ln, bias=b_tiny[:]
                    )
                    nc.scalar.activation(
                        zt[:, :n], zt[:, :n], AF.Exp, bias=b_eight[:],
                        scale=OUT_SCALE / SHARP,
                    )
                    nc.vector.tensor_scalar_add(zt[:, :n], zt[:, :n], 1.0)
                    nc.vector.reciprocal(zt[:, :n], zt[:, :n])
                    c = g * GRP * SUBT
                    nc.sync.dma_start(out_d[:, c : c + n], zt[:, :n])

    nc.compile()

    # Keep one table load per phase: retarget the first load before a
    # Sqrt to the sqrt set, the first before an Exp/Ln to the combined
    # natural-log/exp set, and drop the redundant reloads in between.
    tables = list(get_activation_tables(nc.m.arch).items())
    sqrt_id = next(i for i, (_, fs) in enumerate(tables) if AF.Sqrt in fs)
    nl_id = next(i for i, (_, fs) in enumerate(tables) if {AF.Ln, AF.Exp} <= fs)
    for blk in nc.m.functions[0].blocks:
        cur = None
        pending = []
        for ins in list(blk.instructions):
            if isinstance(ins, mybir.InstLoadActFuncSet):
                pending.append(ins)
            elif isinstance(ins, mybir.InstActivation):
                need = sqrt_id if ins.func == AF.Sqrt else nl_id
                if pending:
                    if need != cur:
                        pending[0].act_func_set_id = need
                        for l in pending[1:]:
                            blk.instructions.remove(l)
                        cur = need
                    else:
                        for l in pending:
                            blk.instructions.remove(l)
                    pending = []
                else:
                    assert cur == need, "activation without table load"
        for l in pending:
            blk.instructions.remove(l)

    return nc, plan


def kernel(control_points: np.ndarray, pixel_grid: np.ndarray) -> np.ndarray:
    control_points = np.asarray(control_points, dtype=np.float32)
    pixel_grid = np.asarray(pixel_grid, dtype=np.float32)

    pts64 = _bezier_points(control_points)
    q64 = pts64.astype(np.float32).astype(np.float64)  # the fp32 values, exactly

    # ---- block geometry from the actual pixel grid ----
    pg = pixel_grid.reshape(SIZE, SIZE, 2)
    pblk = pg.reshape(NB, BLK, NB, BLK, 2).transpose(0, 2, 1, 3, 4)
    pblk = np.ascontiguousarray(pblk).reshape(NBLOCKS, PXB, 2).astype(np.float64)
    bxmin = pblk[:, :, 0].min(1)
    bxmax = pblk[:, :, 0].max(1)
    bymin = pblk[:, :, 1].min(1)
    bymax = pblk[:, :, 1].max(1)
    ccx = 0.5 * (bxmin + bxmax)
    ccy = 0.5 * (bymin + bymax)

    dx = np.maximum(np.maximum(bxmin[:, None] - q64[None, :, 0],
                               q64[None, :, 0] - bxmax[:, None]), 0.0)
    dy = np.maximum(np.maximum(bymin[:, None] - q64[None, :, 1],
                               q64[None, :, 1] - bymax[:, None]), 0.0)
    dc_min = np.sqrt((ccx[:, None] - q64[None, :, 0]) ** 2
                     + (ccy[:, None] - q64[None, :, 1]) ** 2).min(1)
    r_b = np.minimum(CUTOFF, dc_min + HALFDIAG + DELTA)
    cand = dx * dx + dy * dy < (r_b[:, None] + 5e-4) ** 2  # [NBLOCKS, 512]
    kcnt = cand.sum(1)
    nonempty = np.flatnonzero(kcnt > 0)
    kpad = (((kcnt + PADG - 1) // PADG) * PADG).astype(int)

    img = np.ones(SIZE * SIZE, dtype=np.float32)
    if len(nonempty) == 0:
        return img.reshape(1, SIZE, SIZE)

    # ---- deal blocks (sorted desc by padded count) round-robin ----
    order = nonempty[np.argsort(-kpad[nonempty], kind="stable")]
    nslots = (len(order) + NCORES - 1) // NCORES
    core_blocks = [order[c::NCORES] for c in range(NCORES)]
    k_sched = tuple(int(kpad[order[NCORES * i]]) for i in range(nslots))

    if k_sched not in _prog_cache:
        _prog_cache.clear()
        _prog_cache[k_sched] = _build_program(k_sched)
    nc, plan = _prog_cache[k_sched]

    # ---- shared per-block pix rows (block-centered, bf16 limbs) ----
    # rows: [1,1,1, -2x1,-2x1,-2x2, -2y1,-2y1,-2y2, pn1,pn2] pairing mov
    #       [qn1,qn2,qn3, qx1,qx2,qx1, qy1,qy2,qy1, 1,1]
    c_ne = np.stack([ccx[order], ccy[order]], axis=1)  # [n_live, 2]
    prel = pblk[order] - c_ne[:, None, :]  # [n_live, 256, 2]
    x1, x2 = _split2(prel[:, :, 0])
    y1, y2 = _split2(prel[:, :, 1])
    pn1, pn2 = _split2(prel[:, :, 0] ** 2 + prel[:, :, 1] ** 2)
    ones = np.ones_like(x1)
    pix_ne = np.stack(
        [ones, ones, ones,
         -2.0 * x1, -2.0 * x1, -2.0 * x2,
         -2.0 * y1, -2.0 * y1, -2.0 * y2,
         pn1, pn2], axis=1,
    ).astype(np_bf16)  # [n_live, 11, 256]

    # ---- per-core input arrays (block-diagonal matmul layout) ----
    groups = plan["groups"]
    batches = plan["batches"]
    pixcol = plan["pixcol"]
    movcol = plan["movcol"]
    in_maps = []
    for c in range(NCORES):
        blks = core_blocks[c]
        inp = np.zeros((KROWS * MMPACK, plan["totc"]), dtype=np_bf16)
        movs = {}  # slot -> [11, P] block (shared by both subtiles)
        for i in range(len(blks)):
            gi_b = NCORES * i + c
            b = blks[i]
            P = plan["lifted"][i]
            idx = np.flatnonzero(cand[b])
            qrel = q64[idx] - c_ne[gi_b][None, :]
            qx1, qx2 = _split2(qrel[:, 0])
            qy1, qy2 = _split2(qrel[:, 1])
            qn1, qn2, qn3 = _split3(qrel[:, 0] ** 2 + qrel[:, 1] ** 2)
            o1 = np.ones_like(qx1)
            m = np.zeros((KROWS, P), dtype=np_bf16)
            m[:, : len(idx)] = np.stack(
                [qn1, qn2, qn3, qx1, qx2, qx1, qy1, qy2, qy1, o1, o1])
            # far dummy candidate (qrel=(4,4): d>=5.6, exp underflows to 0)
            if P > len(idx):
                dn1, dn2, dn3 = _split3(np.array([32.0]))
                d1, d2 = _split2(np.array([4.0]))
                m[:, len(idx):] = np.array(
                    [dn1[0], dn2[0], dn3[0], d1[0], d2[0], d1[0],
                     d1[0], d2[0], d1[0], 1.0, 1.0], dtype=np_bf16)[:, None]
            movs[i] = m
        for g, (bi, bank, r0, F, P) in enumerate(groups):
            start = batches[bi][0]
            for j in range(F):
                r = r0 + j
                slot = start + r // SUBT
                st = r % SUBT
                i = slot  # per-core slot index
                if i >= len(blks):
                    continue  # dummy slot: zeros are safe (d^2 = 0)
                gi_b = NCORES * i + c
                rows = slice(KROWS * j, KROWS * (j + 1))
                inp[rows, pixcol[g] : pixcol[g] + 128] = (
                    pix_ne[gi_b][:, st * 128 : (st + 1) * 128])
                inp[rows, movcol[g] + j * P : movcol[g] + (j + 1) * P] = movs[i]
        in_maps.append({"inp": inp})

    global _last_in_maps
    _last_in_maps = in_maps
    res = run_bass_kernel_spmd(nc, in_maps, core_ids=list(range(NCORES)))

    # ---- unshard: scatter block results back into the image ----
    by, bx = np.meshgrid(np.arange(NB), np.arange(NB), indexing="ij")
    lr, lc = np.meshgrid(np.arange(BLK), np.arange(BLK), indexing="ij")
    flat = ((by.reshape(-1, 1) * BLK + lr.reshape(-1)[None, :]) * SIZE
            + bx.reshape(-1, 1) * BLK + lc.reshape(-1)[None, :])  # [NBLOCKS, PXB]
    for c in range(NCORES):
        blks = core_blocks[c]
        o = res.results[c]["out"].T.reshape(nslots, PXB)  # [nslots, 256]
        img[flat[blks]] = o[: len(blks)]
    return img.reshape(1, SIZE, SIZE)


# revision 15
# speedup vs baseline: 2.2900x; 1.0573x over previous
"""Trainium2 Bass kernel for nn_BezierGlyph (retrieval_knn).

Math (matching the jax reference):
  pts  = cubic-bezier samples of clip(control_points, 0, 1)   # [512, 2]
  d_ij = |pixel_i - pts_j|
  m_i  = -logsumexp(-256 * d_i:) / 256                        # softmin
  out  = 1 - sigmoid((0.04 - m) * 200)                        # (1, 512, 512)

Strategy (sharding_hint: shard pixels, replicate points):
  * 512x512 pixels in 16x16 blocks (1024). Blocks with no sample point
    within min(0.151, dc_min + 0.070) of their bbox output exactly 1.0f
    and are skipped entirely (host writes the 1.0s). The ~700 live
    blocks are dealt round-robin (sorted by candidate count) over the
    8 cores; the shared SPMD schedule is the slot-wise max.
  * Coordinates are re-centered per block: d^2 = |q-c|^2 + |p-c|^2
    - 2(p-c).(q-c) via an 11-row bf16 limb contraction (2-limb splits
    suffice at these magnitudes; worst-case |noise| ~3e-7, clamped by
    the sqrt bias below). Up to 4 results (128-pixel subtiles) share
    one matmul: their 11-row groups stack on the contraction axis and
    the moving operand is block-diagonal — per-matmul overhead (~60ns)
    dominates at these sizes, so fewer/fatter matmuls win.
  * Two activation passes instead of three:
        v = sqrt(d^2 + 6e-7)        # Sqrt table
        w = exp(-256 * v)           # natural_log_exp table
    The kernel is split into a sqrt phase and an exp phase separated by
    a scheduler-only fence (tc.no_sync_barrier) so the final ACT stream
    needs exactly two table loads (a post-compile pass dedups the
    per-instruction reloads the stock pass inserts).
  * The Sqrt pass reads PSUM directly (no DVE clamp/copy pass); the Exp
    pass runs in place on SBUF; DVE only does the per-result row sums.
  * Inputs arrive as ONE dram tensor in three graduated chunks (first
    two batches, next three, rest) so the first matmul starts early and
    the bulk streams behind it.
  * Per 64-slot group: t = 8 + 0.78125*ln(sum + 1e-37);
    out = 1/(1 + exp(t)), DMA'd untransposed as [128, 2*nslots]
    (host transposes).
"""

import math

import ml_dtypes
import numpy as np

import concourse.bass as bass
import concourse.tile as tile
from concourse import bacc, mybir
from concourse.bass_utils import run_bass_kernel_spmd
from concourse.hw_specs import get_activation_tables

SIZE = 512
N_SAMPLES = 32
N_STROKES = 16
NPTS = N_STROKES * N_SAMPLES  # 512
SHARP = float(N_SAMPLES) * 8.0  # 256
STROKE_WIDTH = 0.04
OUT_SCALE = 8.0 / STROKE_WIDTH  # 200

NCORES = 8
BLK = 16  # block side in pixels
NB = SIZE // BLK  # 32
NBLOCKS = NB * NB  # 1024
PXB = BLK * BLK  # 256 pixels per block
SUBT = PXB // 128  # 2 subtiles of 128 pixels
HALFDIAG = BLK / SIZE * math.sqrt(2.0) / 2.0  # 0.0221
DELTA = 0.048  # points beyond dmin+DELTA are invisible (<=1e-3 out err)
CUTOFF = 0.103 + DELTA  # 0.103 = boring-pixel min_dist bound
PADG = 4  # candidate count granularity
KROWS = 11  # bf16 limb-product rows per result in the contraction
MMPACK = 4  # max results stacked per matmul (44 contraction rows)
GRP = 64  # slots per output group
SQ_BIAS = 6e-7  # clamps fp noise in d^2 (|noise| <~ 3e-7)

f32 = mybir.dt.float32
bf16 = mybir.dt.bfloat16
np_bf16 = ml_dtypes.bfloat16
AF = mybir.ActivationFunctionType

_prog_cache: dict = {}
_last_in_maps = None


def _bezier_points(control_points: np.ndarray) -> np.ndarray:
    """[16,4,2] control points -> [512,2] float64 curve samples."""
    pts = np.clip(control_points.astype(np.float64), 0.0, 1.0)
    t = np.linspace(0.0, 1.0, N_SAMPLES)[None, :, None]
    mt = 1.0 - t
    p0, p1, p2, p3 = (pts[:, k : k + 1, :] for k in range(4))
    cur = mt**3 * p0 + 3 * mt**2 * t * p1 + 3 * mt * t**2 * p2 + t**3 * p3
    return cur.reshape(-1, 2)


def _split2(x: np.ndarray):
    """2-way bf16 limb split (f64 in, 2x bf16 out; remainder ~2^-16 rel)."""
    a = x.astype(np_bf16)
    b = (x - a.astype(np.float64)).astype(np_bf16)
    return a, b


def _split3(x: np.ndarray):
    a = x.astype(np_bf16)
    r = x - a.astype(np.float64)
    b = r.astype(np_bf16)
    c = (r - b.astype(np.float64)).astype(np_bf16)
    return a, b, c


def _plan(k_sched: tuple[int, ...]):
    """Shared host/builder plan for a fixed per-slot candidate schedule.

    Batches: uniform-pitch psum tiles — a batch's 2*nslots results pack
    into one 4-bank tile at pitch P (bank r//rpb, offset (r%rpb)*P,
    rpb = 512//P); batches stay within one output group and stop
    extending when the pitch-lift would exceed 4/3x.

    Matmul groups: up to MMPACK consecutive same-bank results stack
    into one matmul (11 contraction rows each, block-diagonal moving
    operand).

    Input columns: one dram tensor, ordered chunk-by-chunk with each
    chunk's stationary (pix) columns first, then its moving (mov)
    columns, so each chunk is one contiguous DMA.
    """
    nslots = len(k_sched)
    batches = []
    pos = 0
    while pos < nslots:
        P = k_sched[pos]
        rpb = 512 // P
        lim = min((4 * rpb) // SUBT, nslots - pos, GRP - pos % GRP)
        j = 1
        while j < lim and 4 * k_sched[pos + j] >= 3 * P:
            j += 1
        batches.append((pos, j, P))
        pos += j

    lifted = list(k_sched)
    for start, nb, P in batches:
        for j in range(nb):
            lifted[start + j] = P

    # chunk id per batch: 0 = first two, 1 = next three, 2 = rest
    def chunk_of(bi):
        return 0 if bi < 2 else (1 if bi < 5 else 2)

    groups = []  # (bi, bank, r0, F, P)
    for bi, (start, nb, P) in enumerate(batches):
        rpb = 512 // P
        nr = SUBT * nb
        F = min(rpb, MMPACK)
        for b0 in range(0, nr, rpb):
            bend = min(b0 + rpb, nr)
            for r0 in range(b0, bend, F):
                groups.append((bi, b0 // rpb, r0, min(F, bend - r0), P))

    # vt column offset per batch (packed, no gaps)
    voff = []
    tot = 0
    for start, nb, P in batches:
        voff.append(tot)
        tot += SUBT * nb * P

    # Input column layout: DMA bandwidth scales with partition rows, so
    # the logical [44, *] operand blocks are packed into two partition
    # halves (rows 0-43 and 44-87) of an [88, totl] tensor, each chunk
    # split roughly evenly so one DMA covers both halves at full width.
    # (HW pattern rule: APs starting at a non-zero partition may touch
    # at most 32 partitions, so the operands all live at base 0; DMA
    # width is recovered by splitting each chunk across two DGE rings.)
    pix_loc = [None] * len(groups)  # (partition_offset, column)
    mov_loc = [None] * len(groups)
    chunk_slices = []
    cbase = 0
    for c in range(3):
        gs = [gi for gi, g in enumerate(groups) if chunk_of(g[0]) == c]
        start = cbase
        for gi in gs:
            pix_loc[gi] = (0, cbase)
            mov_loc[gi] = (0, cbase + 128)
            cbase += 128 + groups[gi][3] * groups[gi][4]
        if cbase > start:
            chunk_slices.append((start, cbase))
    totl = cbase

    return {
        "nslots": nslots,
        "batches": batches,
        "lifted": lifted,
        "groups": groups,
        "voff": voff,
        "vtot": tot,
        "pix_loc": pix_loc,
        "mov_loc": mov_loc,
        "chunk_slices": chunk_slices,
        "totl": totl,
        "ngroups": (nslots + GRP - 1) // GRP,
    }


def _build_program(k_sched: tuple[int, ...]):
    plan = _plan(k_sched)
    nslots = plan["nslots"]
    batches = plan["batches"]
    groups = plan["groups"]
    voff = plan["voff"]
    pix_loc = plan["pix_loc"]
    mov_loc = plan["mov_loc"]
    totl = plan["totl"]
    PROWS = KROWS * MMPACK  # 44

    nc = bacc.Bacc(None, target_bir_lowering=False, num_swdge_queues=4)

    inp_d = nc.dram_tensor("inp", [PROWS, totl], bf16, kind="ExternalInput")
    out_d = nc.dram_tensor("out", [128, nslots * SUBT], f32, kind="ExternalOutput")

    with tile.TileContext(nc) as tc:
        with (
            tc.tile_pool(name="io", bufs=1) as io,
            tc.tile_pool(name="vtp", bufs=1) as vtp,
            tc.tile_pool(name="acc", bufs=2) as acc,
            tc.tile_pool(name="fin", bufs=2) as fin,
            tc.tile_pool(name="psum", bufs=2, space="PSUM") as psum,
        ):
            inp_all = io.tile([PROWS, totl], bf16)
            # two DGE rings per chunk: DMA rate scales with partition
            # rows (44 here), so halving columns across SP-HWDGE and
            # Pool-SWDGE doubles effective width
            for c0, c1 in plan["chunk_slices"]:
                mid = (c0 + c1) // 2
                nc.sync.dma_start(inp_all[:, c0:mid], inp_d[:, c0:mid])
                nc.gpsimd.dma_start(inp_all[:, mid:c1], inp_d[:, mid:c1])
            b_sqb = io.tile([128, 1], f32)
            nc.vector.memset(b_sqb, SQ_BIAS)
            b_tiny = io.tile([128, 1], f32)
            nc.vector.memset(b_tiny, 1e-37)
            b_eight = io.tile([128, 1], f32)
            nc.vector.memset(b_eight, STROKE_WIDTH * OUT_SCALE)

            vt = vtp.tile([128, plan["vtot"]], f32)

            # ---- phase A: matmuls + sqrt (Sqrt table) ----
            gi = 0
            for bi, (start, nb, P) in enumerate(batches):
                rpb = 512 // P
                nr = SUBT * nb
                pt = psum.tile([128, 4, 512], f32, tag="ps")
                while gi < len(groups) and groups[gi][0] == bi:
                    _, bank, r0, F, _ = groups[gi]
                    o = (r0 % rpb) * P
                    pp, pc = pix_loc[gi]
                    mp, mc = mov_loc[gi]
                    nc.tensor.matmul(
                        pt[:, bank, o : o + F * P],
                        inp_all[pp : pp + KROWS * F, pc : pc + 128],
                        inp_all[mp : mp + KROWS * F, mc : mc + F * P],
                        start=True,
                        stop=True,
                    )
                    gi += 1
                nbf, rem = nr // rpb, nr % rpb
                vo = voff[bi]
                if nbf:
                    nc.scalar.activation(
                        vt[:, vo : vo + nbf * rpb * P].rearrange(
                            "p (b c) -> p b c", c=rpb * P
                        ),
                        pt[:, :nbf, : rpb * P],
                        AF.Sqrt,
                        bias=b_sqb[:],
                    )
                if rem:
                    nc.scalar.activation(
                        vt[:, vo + nbf * rpb * P : vo + nr * P],
                        pt[:, nbf, : rem * P],
                        AF.Sqrt,
                        bias=b_sqb[:],
                    )

            # ACT stream fence: all Sqrts schedule before any Exp/Ln so
            # exactly two table loads survive. Scheduler-only, no sems.
            tc.no_sync_barrier()

            # ---- phase B: exp + row sums + per-group finalization ----
            sums = None
            for bi, (start, nb, P) in enumerate(batches):
                g = start // GRP
                if start % GRP == 0:
                    sums = acc.tile([128, GRP * SUBT], f32, tag="sums")
                nr = SUBT * nb
                vo = voff[bi]
                nc.scalar.activation(
                    vt[:, vo : vo + nr * P],
                    vt[:, vo : vo + nr * P],
                    AF.Exp,
                    scale=-SHARP,
                )
                cs = (start % GRP) * SUBT
                nc.vector.reduce_sum(
                    sums[:, cs : cs + nr],
                    vt[:, vo : vo + nr * P].rearrange("p (r k) -> p r k", k=P),
                    axis=mybir.AxisListType.X,
                )
                last_of_group = (
                    bi + 1 == len(batches) or batches[bi + 1][0] // GRP != g
                )
                if last_of_group:
                    n = min(GRP, nslots - g * GRP) * SUBT
                    zt = fin.tile([128, GRP * SUBT], f32, tag="z")
                    nc.scalar.activation(
                        zt[:, :n], sums[:, :n], AF.Ln, bias=b_tiny[:]
                    )
                    nc.scalar.activation(
                        zt[:, :n], zt[:, :n], AF.Exp, bias=b_eight[:],
                        scale=OUT_SCALE / SHARP,
                    )
                    nc.vector.tensor_scalar_add(zt[:, :n], zt[:, :n], 1.0)
                    nc.vector.reciprocal(zt[:, :n], zt[:, :n])
                    c = g * GRP * SUBT
                    nc.sync.dma_start(out_d[:, c : c + n], zt[:, :n])

    nc.compile()

    # Keep one table load per phase: retarget the first load before a
    # Sqrt to the sqrt set, the first before an Exp/Ln to the combined
    # natural-log/exp set, and drop the redundant reloads in between.
    tables = list(get_activation_tables(nc.m.arch).items())
    sqrt_id = next(i for i, (_, fs) in enumerate(tables) if AF.Sqrt in fs)
    nl_id = next(i for i, (_, fs) in enumerate(tables) if {AF.Ln, AF.Exp} <= fs)
    for blk in nc.m.functions[0].blocks:
        cur = None
        pending = []
        for ins in list(blk.instructions):
            if isinstance(ins, mybir.InstLoadActFuncSet):
                pending.append(ins)
            elif isinstance(ins, mybir.InstActivation):
                need = sqrt_id if ins.func == AF.Sqrt else nl_id
                if pending:
                    if need != cur:
                        pending[0].act_func_set_id = need
                        for l in pending[1:]:
                            blk.instructions.remove(l)
                        cur = need
                    else:
                        for l in pending:
                            blk.instructions.remove(l)
                    pending = []
                else:
                    assert cur == need, "activation without table load"
        for l in pending:
            blk.instructions.remove(l)

    return nc, plan


def kernel(control_points: np.ndarray, pixel_grid: np.ndarray) -> np.ndarray:
    control_points = np.asarray(control_points, dtype=np.float32)
    pixel_grid = np.asarray(pixel_grid, dtype=np.float32)

    pts64 = _bezier_points(control_points)
    q64 = pts64.astype(np.float32).astype(np.float64)  # the fp32 values, exactly

    # ---- block geometry from the actual pixel grid ----
    pg = pixel_grid.reshape(SIZE, SIZE, 2)
    pblk = pg.reshape(NB, BLK, NB, BLK, 2).transpose(0, 2, 1, 3, 4)
    pblk = np.ascontiguousarray(pblk).reshape(NBLOCKS, PXB, 2).astype(np.float64)
    bxmin = pblk[:, :, 0].min(1)
    bxmax = pblk[:, :, 0].max(1)
    bymin = pblk[:, :, 1].min(1)
    bymax = pblk[:, :, 1].max(1)
    ccx = 0.5 * (bxmin + bxmax)
    ccy = 0.5 * (bymin + bymax)

    dx = np.maximum(np.maximum(bxmin[:, None] - q64[None, :, 0],
                               q64[None, :, 0] - bxmax[:, None]), 0.0)
    dy = np.maximum(np.maximum(bymin[:, None] - q64[None, :, 1],
                               q64[None, :, 1] - bymax[:, None]), 0.0)
    dc_min = np.sqrt((ccx[:, None] - q64[None, :, 0]) ** 2
                     + (ccy[:, None] - q64[None, :, 1]) ** 2).min(1)
    r_b = np.minimum(CUTOFF, dc_min + HALFDIAG + DELTA)
    cand = dx * dx + dy * dy < (r_b[:, None] + 5e-4) ** 2  # [NBLOCKS, 512]
    kcnt = cand.sum(1)
    nonempty = np.flatnonzero(kcnt > 0)
    kpad = (((kcnt + PADG - 1) // PADG) * PADG).astype(int)

    img = np.ones(SIZE * SIZE, dtype=np.float32)
    if len(nonempty) == 0:
        return img.reshape(1, SIZE, SIZE)

    # ---- deal blocks (sorted desc by padded count) round-robin ----
    order = nonempty[np.argsort(-kpad[nonempty], kind="stable")]
    nslots = (len(order) + NCORES - 1) // NCORES
    core_blocks = [order[c::NCORES] for c in range(NCORES)]
    k_sched = tuple(int(kpad[order[NCORES * i]]) for i in range(nslots))

    if k_sched not in _prog_cache:
        _prog_cache.clear()
        _prog_cache[k_sched] = _build_program(k_sched)
    nc, plan = _prog_cache[k_sched]

    # ---- shared per-block pix rows (block-centered, bf16 limbs) ----
    # rows: [1,1,1, -2x1,-2x1,-2x2, -2y1,-2y1,-2y2, pn1,pn2] pairing mov
    #       [qn1,qn2,qn3, qx1,qx2,qx1, qy1,qy2,qy1, 1,1]
    c_ne = np.stack([ccx[order], ccy[order]], axis=1)  # [n_live, 2]
    prel = pblk[order] - c_ne[:, None, :]  # [n_live, 256, 2]
    x1, x2 = _split2(prel[:, :, 0])
    y1, y2 = _split2(prel[:, :, 1])
    pn1, pn2 = _split2(prel[:, :, 0] ** 2 + prel[:, :, 1] ** 2)
    ones = np.ones_like(x1)
    pix_ne = np.stack(
        [ones, ones, ones,
         -2.0 * x1, -2.0 * x1, -2.0 * x2,
         -2.0 * y1, -2.0 * y1, -2.0 * y2,
         pn1, pn2], axis=1,
    ).astype(np_bf16)  # [n_live, 11, 256]

    # ---- per-core input arrays (block-diagonal matmul layout) ----
    groups = plan["groups"]
    batches = plan["batches"]
    pix_loc = plan["pix_loc"]
    mov_loc = plan["mov_loc"]
    in_maps = []
    for c in range(NCORES):
        blks = core_blocks[c]
        inp = np.zeros((KROWS * MMPACK, plan["totl"]), dtype=np_bf16)
        movs = {}  # slot -> [11, P] block (shared by both subtiles)
        for i in range(len(blks)):
            gi_b = NCORES * i + c
            b = blks[i]
            P = plan["lifted"][i]
            idx = np.flatnonzero(cand[b])
            qrel = q64[idx] - c_ne[gi_b][None, :]
            qx1, qx2 = _split2(qrel[:, 0])
            qy1, qy2 = _split2(qrel[:, 1])
            qn1, qn2, qn3 = _split3(qrel[:, 0] ** 2 + qrel[:, 1] ** 2)
            o1 = np.ones_like(qx1)
            m = np.zeros((KROWS, P), dtype=np_bf16)
            m[:, : len(idx)] = np.stack(
                [qn1, qn2, qn3, qx1, qx2, qx1, qy1, qy2, qy1, o1, o1])
            # far dummy candidate (qrel=(4,4): d>=5.6, exp underflows to 0)
            if P > len(idx):
                dn1, dn2, dn3 = _split3(np.array([32.0]))
                d1, d2 = _split2(np.array([4.0]))
                m[:, len(idx):] = np.array(
                    [dn1[0], dn2[0], dn3[0], d1[0], d2[0], d1[0],
                     d1[0], d2[0], d1[0], 1.0, 1.0], dtype=np_bf16)[:, None]
            movs[i] = m
        for g, (bi, bank, r0, F, P) in enumerate(groups):
            start = batches[bi][0]
            pp, pc = pix_loc[g]
            mp, mc = mov_loc[g]
            for j in range(F):
                r = r0 + j
                slot = start + r // SUBT
                st = r % SUBT
                i = slot  # per-core slot index
                if i >= len(blks):
                    continue  # dummy slot: zeros are safe (d^2 = 0)
                gi_b = NCORES * i + c
                inp[pp + KROWS * j : pp + KROWS * (j + 1), pc : pc + 128] = (
                    pix_ne[gi_b][:, st * 128 : (st + 1) * 128])
                inp[mp + KROWS * j : mp + KROWS * (j + 1),
                    mc + j * P : mc + (j + 1) * P] = movs[i]
        in_maps.append({"inp": inp})

    global _last_in_maps
    _last_in_maps = in_maps
    res = run_bass_kernel_spmd(nc, in_maps, core_ids=list(range(NCORES)))

    # ---- unshard: scatter block results back into the image ----
    by, bx = np.meshgrid(np.arange(NB), np.arange(NB), indexing="ij")
    lr, lc = np.meshgrid(np.arange(BLK), np.arange(BLK), indexing="ij")
    flat = ((by.reshape(-1, 1) * BLK + lr.reshape(-1)[None, :]) * SIZE
            + bx.reshape(-1, 1) * BLK + lc.reshape(-1)[None, :])  # [NBLOCKS, PXB]
    for c in range(NCORES):
        blks = core_blocks[c]
        o = res.results[c]["out"].T.reshape(nslots, PXB)  # [nslots, 256]
        img[flat[blks]] = o[: len(blks)]
    return img.reshape(1, SIZE, SIZE)
